# revision 2
# baseline (speedup 1.0000x reference)
"""GCN message-passing kernel for 8 Trainium2 NeuronCores (Bass/Tile).

Computation:  out = (segment_sum(relu(x@W1+b1)[edge_src], edge_dst)) @ W2 + b2

Sharding: destination nodes are partitioned across the 8 cores
(degree-balanced); the small 128x128 weights are replicated; each core
computes the full hidden table h = relu(x@W1+b1) itself (replicated compute,
no collectives), then gathers h rows for the edges whose destination it owns
and segment-sums them with one-hot matmuls accumulated in PSUM.

All index manipulation (edge sort/bucketing, permutations) happens on the
host; all FLOPs and all irregular memory traffic happen on device.
"""

import os
import sys

sys.path.insert(0, "/opt/trn_rl_repo")

import numpy as np

import bass_rust
import concourse.bass as bass
import concourse.bacc as bacc
import concourse.mybir as mybir
import concourse.tile as tile_mod
from concourse.tile import TileContext
from concourse.bass_utils import run_bass_kernel_spmd
from concourse import library_config
from concourse._compat import cdiv

NCORES = 8
D = 128
P = 128
NQUART = 4           # src-range classes (int16 gather index reach is 32767 rows)
BATCH_BLOCKS = 6     # dst blocks per gather batch
SWDGE_QUEUES = 4

_PATCHED = False


def _patch_tile_drain():
    """This walrus build only accepts ONE sync-wait on a CTRL (Drain)
    instruction; Tile's end-of-kernel drain carries one wait per DMA sem
    lane.  Split the waits across multiple drain instructions."""
    global _PATCHED
    if _PATCHED:
        return
    _PATCHED = True

    def _patched_dab(self, tick_clock, wait_clock):
        nc = self.nc
        from concourse.vector_clock import ScopedClock

        drain_inst = nc.sync.drain()
        wait_clock.add_sem_waits(
            drain_inst.ins, ScopedClock({None: tick_clock.global_clock})
        )
        si = drain_inst.ins.sync_info
        if si is not None and si.on_wait is not None and len(si.on_wait) > 1:
            waits = list(si.on_wait)
            drain_inst.ins.sync_info = bass_rust.SyncInfo(
                on_wait=[waits[0]], on_update=list(si.on_update or [])
            )
            for w in waits[1:]:
                extra = nc.sync.drain()
                extra.ins.sync_info = bass_rust.SyncInfo(on_wait=[w], on_update=[])
        nc.all_engine_barrier()
        assert self.sems is not None
        popped = nc._tile_sem_poison_stack.pop()
        assert popped is self._sem_poison
        nc.clear_and_free_semaphores(list(self.sems.allocated().values()))
        nc.all_engine_barrier()

    tile_mod.TileContext._drain_and_barrier = _patched_dab


def _assign_nodes(deg, n_parts, part_capacity):
    """Greedy balance nodes into n_parts parts (<=part_capacity nodes each),
    minimizing max edge-sum.  Returns part id per node."""
    order = np.argsort(-deg, kind="stable")
    part = np.empty(len(deg), np.int32)
    sums = np.zeros(n_parts, np.int64)
    counts = np.zeros(n_parts, np.int64)
    # vectorized-ish serpentine for speed, then it's near balanced because
    # degrees are tightly concentrated (Binomial).
    # serpentine: chunks of n_parts in alternating order
    idx = 0
    n = len(deg)
    fwd = np.arange(n_parts)
    rev = fwd[::-1]
    pos = 0
    while pos < n:
        chunk = order[pos : pos + n_parts]
        lane = fwd if (idx % 2 == 0) else rev
        for k, node in enumerate(chunk):
            part[node] = lane[k]
        pos += n_parts
        idx += 1
    # fix capacity violations (only when n not divisible): serpentine keeps
    # counts within +-1 automatically for full chunks; trailing chunk may
    # overfill a part past capacity only if n > n_parts*part_capacity (never).
    counts = np.bincount(part, minlength=n_parts)
    assert counts.max() <= part_capacity, (counts.max(), part_capacity)
    return part


def _build_host_plan(x, edge_src, edge_dst, W1, b1, W2, b2):
    N, Dd = x.shape
    E = edge_src.shape[0]
    assert Dd == D
    nodes_per_core = cdiv(N, NCORES)
    nblk = cdiv(nodes_per_core, P)          # blocks per core
    quart = cdiv(N, NQUART)                 # src rows per class table
    assert quart <= 32768, quart
    qpad = cdiv(quart, P) * P               # padded rows of each h quarter

    deg = np.bincount(edge_dst, minlength=N).astype(np.int64)

    # ---- nodes -> cores (destination/data parallel) ----
    core_of = _assign_nodes(deg, NCORES, nodes_per_core)

    # ---- per core: nodes -> blocks of 128, edge-balanced ----
    # block id + offset inside block, per node
    blk_of = np.empty(N, np.int32)
    off_of = np.empty(N, np.int32)
    for c in range(NCORES):
        nodes_c = np.nonzero(core_of == c)[0]
        part = _assign_nodes(deg[nodes_c], nblk, P)
        # offsets: order nodes within each block
        for b in range(nblk):
            members = nodes_c[part == b]
            blk_of[members] = b
            off_of[members] = np.arange(len(members), dtype=np.int32)

    # ---- edges: group by (core, block, class) ----
    e_core = core_of[edge_dst]
    e_blk = blk_of[edge_dst]
    e_cls = (edge_src // quart).astype(np.int32)
    e_srcrel = (edge_src - e_cls * quart).astype(np.int32)
    e_off = off_of[edge_dst]

    # tiles per (block, class) group, uniform across cores
    key = ((e_core.astype(np.int64) * nblk + e_blk) * NQUART + e_cls)
    group_counts = np.bincount(key, minlength=NCORES * nblk * NQUART)
    T = int(cdiv(int(group_counts.max()), P))  # tiles per (block, class)
    cap = T * P

    # batches of blocks
    batches = []
    b0 = 0
    while b0 < nblk:
        batches.append(min(BATCH_BLOCKS, nblk - b0))
        b0 += BATCH_BLOCKS
    nbatch = len(batches)
    max_b = max(batches)
    s_cols = max_b * T * P // 16            # idx columns per call (max)

    # order edges by key then fill slot arrays
    order = np.argsort(key, kind="stable")
    srcrel_sorted = e_srcrel[order]
    off_sorted = e_off[order]
    key_sorted = key[order]
    # slot position within group
    starts = np.zeros(NCORES * nblk * NQUART + 1, np.int64)
    np.cumsum(group_counts, out=starts[1:])
    within = np.arange(E, dtype=np.int64) - starts[key_sorted]

    # per core arrays
    idx_all = np.zeros((NCORES, nbatch, NQUART, 128, s_cols), np.int16)
    dstoff_all = np.full((NCORES, nblk, 128, NQUART * T), -1.0, np.float32)

    # compute per-edge placement:
    #   group (c,b,r): batch k = b // BATCH_BLOCKS, pos j = b % BATCH_BLOCKS
    #   slot within call = j*cap + within ; call = (c,k,r)
    c_ = (key_sorted // (nblk * NQUART)).astype(np.int64)
    b_ = (key_sorted // NQUART) % nblk
    r_ = key_sorted % NQUART
    k_ = b_ // BATCH_BLOCKS
    j_ = b_ % BATCH_BLOCKS
    slot = j_ * cap + within
    # idx wrapped layout: slot i -> [i % 16 (then replicated x8), i // 16]
    idx_flat_index = (((c_ * nbatch + k_) * NQUART + r_) * 128 + (slot % 16)) * s_cols + slot // 16
    idx_all.reshape(-1)[idx_flat_index] = srcrel_sorted.astype(np.int16)
    # dstoff layout: [core, blk, p, col] ; matmul tile column = r*T + t,
    # partition = slot_in_block % 128 where slot_in_block = within
    col_ = r_ * T + within // P
    doff_flat_index = ((c_ * nblk + b_) * 128 + (within % P)) * (NQUART * T) + col_
    dstoff_all.reshape(-1)[doff_flat_index] = off_sorted.astype(np.float32)
    # replicate idx wrap block to partitions 16..127 (ucode cores each read
    # their own 16-partition group)
    idx_all[:, :, :, 16:, :] = np.tile(idx_all[:, :, :, :16, :], (1, 1, 1, 7, 1))

    # ---- phase-1 xT tiles: per quarter, contiguous [fi, n] tiles ----
    tiles_per_q = qpad // P
    xT_tiles = np.zeros((NQUART * tiles_per_q, P, P), np.float32)
    for r in range(NQUART):
        lo = r * quart
        hi = min(N, lo + quart)
        xq = x[lo:hi]                        # [rows, 128]
        for t in range(tiles_per_q):
            a = t * P
            bnd = min(xq.shape[0], a + P)
            if a >= bnd:
                break
            xT_tiles[r * tiles_per_q + t, :, : bnd - a] = xq[a:bnd].T

    # ---- output unpermutation map ----
    # out_perm row (c, b*128+o) -> node id
    inv = np.zeros((NCORES, nblk * P), np.int64)
    nodes = np.arange(N, dtype=np.int64)
    inv_index = core_of.astype(np.int64) * (nblk * P) + blk_of * P + off_of
    inv.reshape(-1)[inv_index] = nodes
    valid = np.zeros((NCORES, nblk * P), bool)
    valid.reshape(-1)[inv_index] = True

    plan = dict(
        N=N, E=E, nodes_per_core=nodes_per_core, nblk=nblk, quart=quart,
        qpad=qpad, T=T, cap=cap, batches=batches, nbatch=nbatch, max_b=max_b,
        s_cols=s_cols, tiles_per_q=tiles_per_q,
        idx_all=idx_all, dstoff_all=dstoff_all, xT_tiles=xT_tiles,
        inv=inv, valid=valid,
    )
    return plan


def _build_program(plan):
    _patch_tile_drain()
    nblk = plan["nblk"]
    T = plan["T"]
    qpad = plan["qpad"]
    tiles_per_q = plan["tiles_per_q"]
    batches = plan["batches"]
    nbatch = plan["nbatch"]
    max_b = plan["max_b"]
    s_cols = plan["s_cols"]

    nc = bacc.Bacc("TRN2", debug=False, num_swdge_queues=SWDGE_QUEUES)
    f32 = mybir.dt.float32

    xT_t = nc.dram_tensor("xt", [NQUART * tiles_per_q, P, P], f32, kind="ExternalInput")
    idx_t = nc.dram_tensor("idx", [nbatch, NQUART, 128, s_cols], mybir.dt.int16, kind="ExternalInput")
    doff_t = nc.dram_tensor("doff", [nblk, 128, NQUART * T], f32, kind="ExternalInput")
    w1_t = nc.dram_tensor("w1", [P, P], f32, kind="ExternalInput")
    w2_t = nc.dram_tensor("w2", [P, P], f32, kind="ExternalInput")
    b1_t = nc.dram_tensor("b1", [1, P], f32, kind="ExternalInput")
    b2_t = nc.dram_tensor("b2", [1, P], f32, kind="ExternalInput")
    iota_t = nc.dram_tensor("iota", [P, P], f32, kind="ExternalInput")
    ones_t = nc.dram_tensor("ones", [1, P], f32, kind="ExternalInput")
    out_t = nc.dram_tensor("out", [nblk * P, P], f32, kind="ExternalOutput")

    with TileContext(nc) as tc:
        nc.gpsimd.load_library(library_config.mlp)
        with (
            tc.tile_pool(name="const", bufs=1) as constp,
            tc.tile_pool(name="dram", bufs=1, space="DRAM") as dramp,
            tc.tile_pool(name="xt", bufs=3) as xtp,
            tc.tile_pool(name="hp", bufs=3) as hp,
            tc.tile_pool(name="p1", bufs=2, space="PSUM") as p1,
            tc.tile_pool(name="idxp", bufs=8) as idxp,
            tc.tile_pool(name="arena", bufs=2) as arenap,
            tc.tile_pool(name="dop", bufs=3) as dop,
            tc.tile_pool(name="ohp", bufs=4) as ohp,
            tc.tile_pool(name="p2", bufs=4, space="PSUM") as p2,
            tc.tile_pool(name="agp", bufs=3) as agp,
            tc.tile_pool(name="p3", bufs=2, space="PSUM") as p3,
            tc.tile_pool(name="outp", bufs=3) as outp,
        ):
            w1s = constp.tile([P, P], f32, tag="w1")
            nc.sync.dma_start(w1s[:], w1_t[:])
            w2s = constp.tile([P, P], f32, tag="w2")
            nc.sync.dma_start(w2s[:], w2_t[:])
            b1s = constp.tile([1, P], f32, tag="b1")
            nc.sync.dma_start(b1s[:], b1_t[:])
            b2s = constp.tile([1, P], f32, tag="b2")
            nc.sync.dma_start(b2s[:], b2_t[:])
            iotas = constp.tile([P, P], f32, tag="iota")
            nc.sync.dma_start(iotas[:], iota_t[:])
            oness = constp.tile([1, P], f32, tag="ones")
            nc.sync.dma_start(oness[:], ones_t[:])

            h_q = [dramp.tile([qpad, P], f32, name=f"hq{r}", tag=f"hq{r}") for r in range(NQUART)]

            # ---- phase 1: h = relu(x@W1 + b1), one quarter at a time ----
            for r in range(NQUART):
                for t in range(tiles_per_q):
                    xt = xtp.tile([P, P], f32, tag="xt")
                    nc.sync.dma_start(xt[:], xT_t[r * tiles_per_q + t])
                    ph = p1.tile([P, P], f32, tag="p1")
                    nc.tensor.matmul(ph[:], xt[:], w1s[:], start=True, stop=False)
                    nc.tensor.matmul(ph[:], oness[:], b1s[:], start=False, stop=True)
                    hs = hp.tile([P, P], f32, tag="hs")
                    nc.vector.tensor_scalar_max(hs[:], ph[:], 0.0)
                    nc.sync.dma_start(h_q[r][t * P : (t + 1) * P, :], hs[:])

            # ---- phase 2/3 ----
            blk0 = 0
            for k in range(nbatch):
                B = batches[k]
                arenas = []
                for r in range(NQUART):
                    it = idxp.tile([128, s_cols], mybir.dt.int16, tag="idx")
                    nc.sync.dma_start(it[:], idx_t[k, r])
                    ar = arenap.tile([128, max_b * T, P], f32, name=f"ar{r}", tag=f"ar{r}")
                    nidx = B * T * P
                    nc.gpsimd.dma_gather(
                        ar[:, : B * T, :], h_q[r][:], it[:],
                        nidx, nidx, P,
                        single_packet=False, queue_num=r % SWDGE_QUEUES,
                    )
                    arenas.append(ar)
                for j in range(B):
                    blk = blk0 + j
                    do = dop.tile([128, NQUART * T], f32, tag="do")
                    nc.sync.dma_start(do[:], doff_t[blk])
                    pa = p2.tile([P, P], f32, tag="p2")
                    ntile = NQUART * T
                    for jj in range(ntile):
                        r, t = jj // T, jj % T
                        oh = ohp.tile([P, P], f32, tag="oh")
                        nc.vector.tensor_scalar(
                            oh[:], iotas[:], do[:, jj : jj + 1], None,
                            op0=mybir.AluOpType.is_equal,
                        )
                        nc.tensor.matmul(
                            pa[:], arenas[r][:, j * T + t, :], oh[:],
                            start=(jj == 0), stop=(jj == ntile - 1),
                        )
                    ag = agp.tile([P, P], f32, tag="ag")
                    nc.vector.tensor_copy(ag[:], pa[:])
                    po = p3.tile([P, P], f32, tag="p3")
                    nc.tensor.matmul(po[:], ag[:], w2s[:], start=True, stop=False)
                    nc.tensor.matmul(po[:], oness[:], b2s[:], start=False, stop=True)
                    ot = outp.tile([P, P], f32, tag="ot")
                    nc.scalar.copy(ot[:], po[:])
                    nc.sync.dma_start(out_t[blk * P : (blk + 1) * P, :], ot[:])
                blk0 += B

    nc.compile()
    return nc


def _run(plan, W1, b1, W2, b2, trace=False):
    nc = _build_program(plan)
    iota2d = np.tile(np.arange(P, dtype=np.float32)[None, :], (P, 1))
    ones = np.ones((1, P), np.float32)
    in_maps = []
    for c in range(NCORES):
        in_maps.append({
            "xt": plan["xT_tiles"],
            "idx": plan["idx_all"][c],
            "doff": plan["dstoff_all"][c],
            "w1": W1.astype(np.float32),
            "w2": W2.astype(np.float32),
            "b1": b1.reshape(1, P).astype(np.float32),
            "b2": b2.reshape(1, P).astype(np.float32),
            "iota": iota2d,
            "ones": ones,
        })
    res = run_bass_kernel_spmd(nc, in_maps, core_ids=list(range(NCORES)), trace=trace)
    return res


def kernel(x, edge_src, edge_dst, W1, b1, W2, b2, _trace=False, _ret_stats=False):
    x = np.asarray(x, np.float32)
    edge_src = np.asarray(edge_src).astype(np.int64)
    edge_dst = np.asarray(edge_dst).astype(np.int64)
    plan = _build_host_plan(x, edge_src, edge_dst, W1, b1, W2, b2)
    res = _run(plan, np.asarray(W1), np.asarray(b1), np.asarray(W2), np.asarray(b2),
               trace=_trace)
    N = plan["N"]
    out = np.zeros((N, D), np.float32)
    for c in range(NCORES):
        o = res.results[c]["out"]            # [nblk*128, 128]
        v = plan["valid"][c]
        out[plan["inv"][c][v]] = o[v]
    if _ret_stats:
        return out, res
    return out


# revision 4
# speedup vs baseline: 1.7262x; 1.7262x over previous
"""GCN message-passing kernel for 8 Trainium2 NeuronCores (Bass/Tile).

Computation:  out = (segment_sum(relu(x@W1+b1)[edge_src], edge_dst)) @ W2 + b2

Sharding: destination nodes are partitioned across the 8 cores
(degree-balanced); the small 128x128 weights are replicated; each core
computes the full hidden table h = relu(x@W1+b1) itself (replicated compute,
no collectives), then gathers h rows for the edges whose destination it owns
(SWDGE dma_gather on 4 queues) and segment-sums them with one-hot matmuls
accumulated in PSUM; finally multiplies by W2 with the PSUM result as the
stationary operand so the output lands node-major.

All index manipulation (edge sort/bucketing, permutations) happens on the
host; all FLOPs and all irregular memory traffic happen on device.
"""

import os
import sys

sys.path.insert(0, "/opt/trn_rl_repo")

import numpy as np

import bass_rust
import concourse.bass as bass
import concourse.bacc as bacc
import concourse.mybir as mybir
import concourse.tile as tile_mod
from concourse.tile import TileContext
from concourse.bass_utils import run_bass_kernel_spmd
from concourse import library_config
from concourse._compat import cdiv

NCORES = 8
D = 128
P = 128
NQUART = 4           # src-range classes (int16 gather index reach is 32767 rows)
BATCH_BLOCKS = 6     # dst blocks per gather batch
SWDGE_QUEUES = 4
CHUNK = 7            # phase-1 tiles per DMA chunk (divides qpad/128)

_PATCHED = False


def _patch_tile_drain():
    """This walrus build only accepts ONE sync-wait on a CTRL (Drain)
    instruction; Tile's end-of-kernel drain carries one wait per DMA sem
    lane.  Split the waits across multiple drain instructions."""
    global _PATCHED
    if _PATCHED:
        return
    _PATCHED = True

    def _patched_dab(self, tick_clock, wait_clock):
        nc = self.nc
        from concourse.vector_clock import ScopedClock

        drain_inst = nc.sync.drain()
        wait_clock.add_sem_waits(
            drain_inst.ins, ScopedClock({None: tick_clock.global_clock})
        )
        si = drain_inst.ins.sync_info
        if si is not None and si.on_wait is not None and len(si.on_wait) > 1:
            waits = list(si.on_wait)
            drain_inst.ins.sync_info = bass_rust.SyncInfo(
                on_wait=[waits[0]], on_update=list(si.on_update or [])
            )
            for w in waits[1:]:
                extra = nc.sync.drain()
                extra.ins.sync_info = bass_rust.SyncInfo(on_wait=[w], on_update=[])
        nc.all_engine_barrier()
        assert self.sems is not None
        popped = nc._tile_sem_poison_stack.pop()
        assert popped is self._sem_poison
        nc.clear_and_free_semaphores(list(self.sems.allocated().values()))
        nc.all_engine_barrier()

    tile_mod.TileContext._drain_and_barrier = _patched_dab


def _assign_nodes(deg, n_parts, part_capacity):
    """Serpentine-balance nodes into n_parts parts (<= part_capacity nodes
    each) by degree.  Returns part id per node."""
    order = np.argsort(-deg, kind="stable")
    part = np.empty(len(deg), np.int32)
    n = len(deg)
    fwd = np.arange(n_parts)
    rev = fwd[::-1]
    pos = 0
    row = 0
    while pos < n:
        chunk = order[pos : pos + n_parts]
        lane = fwd if (row % 2 == 0) else rev
        part[chunk] = lane[: len(chunk)]
        pos += n_parts
        row += 1
    counts = np.bincount(part, minlength=n_parts)
    assert counts.max() <= part_capacity, (counts.max(), part_capacity)
    return part


def _build_host_plan(x, edge_src, edge_dst, W1, b1, W2, b2):
    N, Dd = x.shape
    E = edge_src.shape[0]
    assert Dd == D
    nodes_per_core = cdiv(N, NCORES)
    nblk = cdiv(nodes_per_core, P)          # blocks per core
    quart = cdiv(N, NQUART)                 # src rows per class table
    assert quart <= 32768, quart
    qpad = cdiv(quart, P) * P               # padded rows of each h quarter
    tiles_per_q = qpad // P
    assert tiles_per_q % CHUNK == 0 or tiles_per_q < CHUNK, (tiles_per_q, CHUNK)
    ch = CHUNK if tiles_per_q >= CHUNK else tiles_per_q
    chunks_per_q = cdiv(tiles_per_q, ch)

    deg = np.bincount(edge_dst, minlength=N).astype(np.int64)

    # ---- nodes -> cores (destination/data parallel) ----
    core_of = _assign_nodes(deg, NCORES, nodes_per_core)

    # ---- per core: nodes -> blocks of 128, edge-balanced ----
    blk_of = np.empty(N, np.int32)
    off_of = np.empty(N, np.int32)
    for c in range(NCORES):
        nodes_c = np.nonzero(core_of == c)[0]
        part = _assign_nodes(deg[nodes_c], nblk, P)
        for b in range(nblk):
            members = nodes_c[part == b]
            blk_of[members] = b
            off_of[members] = np.arange(len(members), dtype=np.int32)

    # ---- edges: group by (core, block, class) ----
    e_core = core_of[edge_dst]
    e_blk = blk_of[edge_dst]
    e_cls = (edge_src // quart).astype(np.int32)
    e_srcrel = (edge_src - e_cls * quart).astype(np.int32)
    e_off = off_of[edge_dst]

    key = ((e_core.astype(np.int64) * nblk + e_blk) * NQUART + e_cls)
    group_counts = np.bincount(key, minlength=NCORES * nblk * NQUART)
    T = int(cdiv(int(group_counts.max()), P))  # tiles per (block, class)
    cap = T * P

    batches = []
    b0 = 0
    while b0 < nblk:
        batches.append(min(BATCH_BLOCKS, nblk - b0))
        b0 += BATCH_BLOCKS
    nbatch = len(batches)
    max_b = max(batches)
    s_cols = max_b * T * P // 16            # idx columns per call (max)

    order = np.argsort(key, kind="stable")
    srcrel_sorted = e_srcrel[order]
    off_sorted = e_off[order]
    key_sorted = key[order]
    starts = np.zeros(NCORES * nblk * NQUART + 1, np.int64)
    np.cumsum(group_counts, out=starts[1:])
    within = np.arange(E, dtype=np.int64) - starts[key_sorted]

    idx_all = np.zeros((NCORES, nbatch, NQUART, 128, s_cols), np.int16)
    dstoff_all = np.full((NCORES, nblk, 128, NQUART * T), -1.0, np.float32)

    c_ = (key_sorted // (nblk * NQUART)).astype(np.int64)
    b_ = (key_sorted // NQUART) % nblk
    r_ = key_sorted % NQUART
    k_ = b_ // BATCH_BLOCKS
    j_ = b_ % BATCH_BLOCKS
    slot = j_ * cap + within
    idx_flat_index = (((c_ * nbatch + k_) * NQUART + r_) * 128 + (slot % 16)) * s_cols + slot // 16
    idx_all.reshape(-1)[idx_flat_index] = srcrel_sorted.astype(np.int16)
    col_ = r_ * T + within // P
    doff_flat_index = ((c_ * nblk + b_) * 128 + (within % P)) * (NQUART * T) + col_
    dstoff_all.reshape(-1)[doff_flat_index] = off_sorted.astype(np.float32)
    idx_all[:, :, :, 16:, :] = np.tile(idx_all[:, :, :, :16, :], (1, 1, 1, 7, 1))

    # ---- phase-1 xT chunks ----
    # h-table row r of quarter rr is computed by tile t = r % tiles_per_q in
    # partition p = r // tiles_per_q, so a CHUNK-tile group writes per
    # partition `ch` contiguous rows (one 512B*ch descriptor per partition).
    # xT chunk layout: [chunk, fi, i, p] where tile t = chunk*ch + i holds
    # nodes  quarter_base + p*tiles_per_q + t.
    xT_chunks = np.zeros((NQUART * chunks_per_q, P, ch, P), np.float32)
    for rr in range(NQUART):
        lo = rr * quart
        hi = min(N, lo + quart)
        xq = np.zeros((qpad, D), np.float32)
        xq[: hi - lo] = x[lo:hi]
        # [p, t, fi]
        xqv = xq.reshape(P, tiles_per_q, D)
        # -> [t, fi, p]
        xt = np.transpose(xqv, (1, 2, 0))
        # -> [chunk, i, fi, p] -> [chunk, fi, i, p]
        xt = xt.reshape(chunks_per_q, ch, D, P).transpose(0, 2, 1, 3)
        xT_chunks[rr * chunks_per_q : (rr + 1) * chunks_per_q] = xt

    # ---- output unpermutation map ----
    inv = np.zeros((NCORES, nblk * P), np.int64)
    nodes = np.arange(N, dtype=np.int64)
    inv_index = core_of.astype(np.int64) * (nblk * P) + blk_of * P + off_of
    inv.reshape(-1)[inv_index] = nodes
    valid = np.zeros((NCORES, nblk * P), bool)
    valid.reshape(-1)[inv_index] = True

    plan = dict(
        N=N, E=E, nodes_per_core=nodes_per_core, nblk=nblk, quart=quart,
        qpad=qpad, T=T, cap=cap, batches=batches, nbatch=nbatch, max_b=max_b,
        s_cols=s_cols, tiles_per_q=tiles_per_q, ch=ch, chunks_per_q=chunks_per_q,
        idx_all=idx_all, dstoff_all=dstoff_all, xT_chunks=xT_chunks,
        inv=inv, valid=valid,
        has_b1=bool(np.any(np.asarray(b1))), has_b2=bool(np.any(np.asarray(b2))),
    )
    return plan


def _build_program(plan):
    _patch_tile_drain()
    nblk = plan["nblk"]
    T = plan["T"]
    qpad = plan["qpad"]
    tiles_per_q = plan["tiles_per_q"]
    ch = plan["ch"]
    chunks_per_q = plan["chunks_per_q"]
    batches = plan["batches"]
    nbatch = plan["nbatch"]
    max_b = plan["max_b"]
    s_cols = plan["s_cols"]
    has_b1 = plan["has_b1"]
    has_b2 = plan["has_b2"]
    NT = NQUART * T

    nc = bacc.Bacc("TRN2", debug=False, num_swdge_queues=SWDGE_QUEUES)
    f32 = mybir.dt.float32

    xT_t = nc.dram_tensor("xt", [NQUART * chunks_per_q, P, ch * P], f32, kind="ExternalInput")
    idx_t = nc.dram_tensor("idx", [nbatch, NQUART, 128, s_cols], mybir.dt.int16, kind="ExternalInput")
    doff_t = nc.dram_tensor("doff", [nblk, 128, NT], f32, kind="ExternalInput")
    w1_t = nc.dram_tensor("w1", [P, P], f32, kind="ExternalInput")
    w2_t = nc.dram_tensor("w2", [P, P], f32, kind="ExternalInput")
    b1_t = nc.dram_tensor("b1", [1, P], f32, kind="ExternalInput")
    b2_t = nc.dram_tensor("b2", [1, P], f32, kind="ExternalInput")
    iota_t = nc.dram_tensor("iota", [P, NT * P], f32, kind="ExternalInput")
    ones_t = nc.dram_tensor("ones", [1, P], f32, kind="ExternalInput")
    out_t = nc.dram_tensor("out", [nblk * P, P], f32, kind="ExternalOutput")

    with TileContext(nc) as tc:
        nc.gpsimd.load_library(library_config.mlp)
        with (
            tc.tile_pool(name="const", bufs=1) as constp,
            tc.tile_pool(name="dram", bufs=1, space="DRAM") as dramp,
            tc.tile_pool(name="xt", bufs=3) as xtp,
            tc.tile_pool(name="hp", bufs=3) as hp,
            tc.tile_pool(name="p1", bufs=3, space="PSUM") as p1,
            tc.tile_pool(name="idxp", bufs=8) as idxp,
            tc.tile_pool(name="arena", bufs=2) as arenap,
            tc.tile_pool(name="dop", bufs=3) as dop,
            tc.tile_pool(name="ohp", bufs=2) as ohp,
            tc.tile_pool(name="p2", bufs=3, space="PSUM") as p2,
            tc.tile_pool(name="agp", bufs=3) as agp,
            tc.tile_pool(name="p3", bufs=2, space="PSUM") as p3,
            tc.tile_pool(name="outp", bufs=3) as outp,
        ):
            w1s = constp.tile([P, P], f32, tag="w1")
            nc.sync.dma_start(w1s[:], w1_t[:])
            w2s = constp.tile([P, P], f32, tag="w2")
            nc.sync.dma_start(w2s[:], w2_t[:])
            b1s = constp.tile([1, P], f32, tag="b1")
            nc.sync.dma_start(b1s[:], b1_t[:])
            b2s = constp.tile([1, P], f32, tag="b2")
            nc.sync.dma_start(b2s[:], b2_t[:])
            iotas = constp.tile([P, NT * P], f32, tag="iota")
            nc.sync.dma_start(iotas[:], iota_t[:])
            oness = constp.tile([1, P], f32, tag="ones")
            nc.sync.dma_start(oness[:], ones_t[:])

            h_q = [dramp.tile([qpad, P], f32, name=f"hq{r}", tag=f"hq{r}") for r in range(NQUART)]

            # ---- phase 1: h = relu(x@W1 + b1), one quarter at a time ----
            for r in range(NQUART):
                hqv = h_q[r].rearrange("(p t) f -> p t f", t=tiles_per_q)
                for c in range(chunks_per_q):
                    xb = xtp.tile([P, ch * P], f32, tag="xt")
                    nc.sync.dma_start(xb[:], xT_t[r * chunks_per_q + c])
                    hb = hp.tile([P, ch * P], f32, tag="hs")
                    for i in range(ch):
                        ph = p1.tile([P, P], f32, tag="p1")
                        if has_b1:
                            nc.tensor.matmul(ph[:], xb[:, i * P : (i + 1) * P], w1s[:], start=True, stop=False)
                            nc.tensor.matmul(ph[:], oness[:], b1s[:], start=False, stop=True)
                        else:
                            nc.tensor.matmul(ph[:], xb[:, i * P : (i + 1) * P], w1s[:], start=True, stop=True)
                        nc.vector.tensor_scalar_max(hb[:, i * P : (i + 1) * P], ph[:], 0.0)
                    nc.sync.dma_start(hqv[:, c * ch : (c + 1) * ch, :], hb[:].rearrange("p (i f) -> p i f", i=ch))

            # ---- phase 2/3 ----
            blk0 = 0
            for k in range(nbatch):
                B = batches[k]
                arenas = []
                for r in range(NQUART):
                    it = idxp.tile([128, s_cols], mybir.dt.int16, tag="idx")
                    nc.sync.dma_start(it[:], idx_t[k, r])
                    ar = arenap.tile([128, max_b * T, P], f32, name=f"ar{r}", tag=f"ar{r}")
                    nidx = B * T * P
                    nc.gpsimd.dma_gather(
                        ar[:, : B * T, :], h_q[r][:], it[:],
                        nidx, nidx, P,
                        single_packet=False, queue_num=r % SWDGE_QUEUES,
                    )
                    arenas.append(ar)
                for j in range(B):
                    blk = blk0 + j
                    do = dop.tile([128, NT], f32, tag="do")
                    nc.sync.dma_start(do[:], doff_t[blk])
                    oh = ohp.tile([P, NT, P], f32, tag="oh")
                    nc.vector.tensor_tensor(
                        oh[:],
                        do[:, :NT].to_broadcast([P, NT, P]),
                        iotas[:].rearrange("p (j d) -> p j d", j=NT),
                        op=mybir.AluOpType.is_equal,
                    )
                    pa = p2.tile([P, P], f32, tag="p2")
                    for jj in range(NT):
                        r, t = jj // T, jj % T
                        nc.tensor.matmul(
                            pa[:], arenas[r][:, j * T + t, :], oh[:, jj, :],
                            start=(jj == 0), stop=(jj == NT - 1),
                        )
                    ag = agp.tile([P, P], f32, tag="ag")
                    nc.vector.tensor_copy(ag[:], pa[:])
                    po = p3.tile([P, P], f32, tag="p3")
                    if has_b2:
                        nc.tensor.matmul(po[:], ag[:], w2s[:], start=True, stop=False)
                        nc.tensor.matmul(po[:], oness[:], b2s[:], start=False, stop=True)
                    else:
                        nc.tensor.matmul(po[:], ag[:], w2s[:], start=True, stop=True)
                    ot = outp.tile([P, P], f32, tag="ot")
                    nc.scalar.copy(ot[:], po[:])
                    nc.sync.dma_start(out_t[blk * P : (blk + 1) * P, :], ot[:])
                blk0 += B

    nc.compile()
    return nc


def _run(plan, W1, b1, W2, b2, trace=False):
    nc = _build_program(plan)
    NT = NQUART * plan["T"]
    iota_rep = np.tile(np.arange(P, dtype=np.float32)[None, None, :], (P, NT, 1)).reshape(P, NT * P)
    ones = np.ones((1, P), np.float32)
    in_maps = []
    for c in range(NCORES):
        in_maps.append({
            "xt": plan["xT_chunks"].reshape(plan["xT_chunks"].shape[0], P, -1),
            "idx": plan["idx_all"][c],
            "doff": plan["dstoff_all"][c],
            "w1": np.asarray(W1, np.float32).reshape(P, P),
            "w2": np.asarray(W2, np.float32).reshape(P, P),
            "b1": np.asarray(b1, np.float32).reshape(1, P),
            "b2": np.asarray(b2, np.float32).reshape(1, P),
            "iota": iota_rep,
            "ones": ones,
        })
    res = run_bass_kernel_spmd(nc, in_maps, core_ids=list(range(NCORES)), trace=trace)
    return res


def kernel(x, edge_src, edge_dst, W1, b1, W2, b2, _trace=False, _ret_stats=False):
    x = np.asarray(x, np.float32)
    edge_src = np.asarray(edge_src).astype(np.int64)
    edge_dst = np.asarray(edge_dst).astype(np.int64)
    plan = _build_host_plan(x, edge_src, edge_dst, W1, b1, W2, b2)
    res = _run(plan, np.asarray(W1), np.asarray(b1), np.asarray(W2), np.asarray(b2),
               trace=_trace)
    N = plan["N"]
    out = np.zeros((N, D), np.float32)
    for c in range(NCORES):
        o = res.results[c]["out"]            # [nblk*128, 128]
        v = plan["valid"][c]
        out[plan["inv"][c][v]] = o[v]
    if _ret_stats:
        return out, res
    return out
